# revision 4
# baseline (speedup 1.0000x reference)
import os
import sys
from contextlib import ExitStack

import numpy as np

for _p in ("/opt/trn_rl_repo", "/root/.axon_site/_ro/trn_rl_repo"):
    if os.path.isdir(_p) and _p not in sys.path:
        sys.path.insert(0, _p)

import concourse.bass as bass
from concourse import mybir
from concourse.bass_utils import run_bass_kernel_spmd

B, D, C = 32768, 1024, 256
M_CORES = 8
BS = B // M_CORES
P = 128
GQ = 4
N_GROUPS = BS // (P * GQ)
N_SUB = N_GROUPS * GQ
WEIGHT = 0.0005
EPS = 1e-12

F32 = mybir.dt.float32
BF16 = mybir.dt.bfloat16

XSLOTS = 6
XSS = 4
OHB = 8
OHF = 6
N_WARM = 16


def _sub_kind(t, a):
    return a == 3 or (a == 2 and t % 2 == 0)


def _ssq_on_dve(t, a):
    return a == 3


def build_nc(bs=BS):
    Sq = mybir.ActivationFunctionType.Square
    Sqrt = mybir.ActivationFunctionType.Sqrt
    CopyF = mybir.ActivationFunctionType.Copy

    subs = [(t, a) for t in range(N_GROUPS) for a in range(GQ)]
    is_f32 = {s: _sub_kind(*s) for s in subs}
    bidx, fidx = {}, {}
    bseq, fseq = [], []
    for s in subs:
        if is_f32[s]:
            fidx[s] = len(fseq)
            fseq.append(s)
        else:
            bidx[s] = len(bseq)
            bseq.append(s)
    ksub = {s: s[0] * GQ + s[1] for s in subs}
    n_scales_upto = {}
    cnt = 0
    for s in subs:
        if not is_f32[s]:
            cnt += 1
        n_scales_upto[s] = cnt

    nc = bass.Bass()
    x = nc.declare_dram_parameter("x", [bs, D], F32, isOutput=False)
    lab = nc.declare_dram_parameter("labf", [bs], F32, isOutput=False)
    aux = nc.declare_dram_parameter("aux", [P, C + 2], F32, isOutput=False)
    sumsA = nc.declare_dram_parameter("sumsA", [C, D], BF16, isOutput=True)
    sumsB = nc.declare_dram_parameter("sumsB", [C, D], BF16, isOutput=True)

    with ExitStack() as stk:
        en = stk.enter_context
        xt = en(nc.sbuf_tensor([P, XSLOTS, GQ, D], F32))
        xs = en(nc.sbuf_tensor([P, XSS, 3, D], BF16))
        sqscr = en(nc.sbuf_tensor([P, D], BF16))
        vscr = en(nc.sbuf_tensor([P, D], BF16))
        auxs = en(nc.sbuf_tensor([P, C + 2], F32))
        auxb = en(nc.sbuf_tensor([P, C], BF16))
        labf = en(nc.sbuf_tensor([P, N_GROUPS, GQ], F32))
        ssq = en(nc.sbuf_tensor([P, N_SUB], F32))
        nrm = en(nc.sbuf_tensor([P, N_SUB], F32))
        rr = en(nc.sbuf_tensor([P, N_SUB], F32))
        ohb = en(nc.sbuf_tensor([P, OHB, C], BF16))
        ohf = en(nc.sbuf_tensor([P, OHF, C], F32))
        outA = en(nc.sbuf_tensor([P, 2, D], BF16))
        outB = en(nc.sbuf_tensor([P, 2, D], BF16))
        dum = en(nc.sbuf_tensor([P, 2], F32))
        ps = {}
        for h in range(2):
            for mi in range(2):
                for ni in range(2):
                    ps[(h, mi, ni)] = en(
                        nc.psum_tensor(f"ps_{h}{mi}{ni}", [P, 512], F32)
                    )

        s_aux = en(nc.semaphore("s_aux"))
        s_lab = en(nc.semaphore("s_lab"))
        s_x0 = [en(nc.semaphore(f"s_x0_{a}")) for a in range(GQ)]
        s_x = [
            [en(nc.semaphore(f"s_x_{t}_{h}")) for h in range(2)]
            for t in range(1, N_GROUPS)
        ]
        s_act_ssq = en(nc.semaphore("s_act_ssq"))
        s_dve_ssq = en(nc.semaphore("s_dve_ssq"))
        s_act_nrm = en(nc.semaphore("s_act_nrm"))
        s_dve = en(nc.semaphore("s_dve"))
        s_pl_oh = en(nc.semaphore("s_pl_oh"))
        s_xs = en(nc.semaphore("s_xs"))
        s_pe_mm = en(nc.semaphore("s_pe_mm"))
        s_act_out = en(nc.semaphore("s_act_out"))
        s_dve_out = en(nc.semaphore("s_dve_out"))
        s_dma_out = en(nc.semaphore("s_dma_out"))
        block = en(nc.Block(no_gpsimd_drain=True))

        def wait_x(eng, t, half):
            if t == 0:
                eng.wait_ge(s_x0[2 * half + 1], 16)
            else:
                eng.wait_ge(s_x[t - 1][half], 16)

        def wait_x_lane(eng, t, a):
            if t == 0:
                eng.wait_ge(s_x0[a], 16)
            else:
                eng.wait_ge(s_x[t - 1][a // 2], 16)

        @block.sync
        def _(sync):
            sync.dma_start(out=auxs[:, :], in_=aux[:, :]).then_inc(s_aux, 16)
            src0 = x[0 : P * GQ, :].rearrange("(p g) d -> p g d", p=P)
            for a in range(GQ):
                sync.dma_start(out=xt[:, 0, a, :], in_=src0[:, a, :]).then_inc(
                    s_x0[a], 16
                )
            sync.dma_start(
                out=labf[:, :, :],
                in_=lab[0:bs].rearrange("(t p a) -> p t a", t=N_GROUPS, p=P, a=GQ),
            ).then_inc(s_lab, 16)
            for t in range(1, N_GROUPS):
                if t >= XSLOTS:
                    sync.wait_ge(s_pe_mm, GQ * (t - XSLOTS + 1))
                src = x[t * P * GQ : (t + 1) * P * GQ, :].rearrange(
                    "(p g) d -> p g d", p=P
                )
                sync.dma_start(
                    out=xt[:, t % XSLOTS, 0:2, :], in_=src[:, 0:2, :]
                ).then_inc(s_x[t - 1][0], 16)
                sync.dma_start(
                    out=xt[:, t % XSLOTS, 2:4, :], in_=src[:, 2:4, :]
                ).then_inc(s_x[t - 1][1], 16)
            sync.wait_ge(s_act_out, 2)
            sync.dma_start(out=sumsA[0:128, :], in_=outA[:, 0, :]).then_inc(
                s_dma_out, 16
            )
            sync.wait_ge(s_dve_out, 2)
            sync.dma_start(out=sumsA[128:256, :], in_=outA[:, 1, :]).then_inc(
                s_dma_out, 16
            )
            sync.wait_ge(s_act_out, 4)
            sync.dma_start(out=sumsB[0:128, :], in_=outB[:, 0, :]).then_inc(
                s_dma_out, 16
            )
            sync.wait_ge(s_dve_out, 4)
            sync.dma_start(out=sumsB[128:256, :], in_=outB[:, 1, :]).then_inc(
                s_dma_out, 16
            )
            sync.wait_ge(s_dma_out, 64)

        @block.scalar
        def _(scalar):
            zero_bias = auxs[:, C : C + 1]
            eps_bias = auxs[:, C + 1 : C + 2]
            scalar.wait_ge(s_aux, 16)
            scalar.activation(dum[:, 0:1], auxs[:, 0:1], Sq, bias=zero_bias)
            scalar.activation(dum[:, 1:2], auxs[:, 0:1], Sqrt, bias=zero_bias)
            act_ssq_count = 0
            nrm_count = 0
            for t in range(N_GROUPS):
                fine = t == N_GROUPS - 1
                acts = [a for a in range(GQ) if not _ssq_on_dve(t, a)]
                for a in acts:
                    k = t * GQ + a
                    wait_x_lane(scalar, t, a)
                    scalar.activation(
                        sqscr[:, :],
                        xt[:, t % XSLOTS, a, :],
                        Sq,
                        bias=zero_bias,
                        accum_out=ssq[:, k : k + 1],
                    ).then_inc(s_act_ssq, 1)
                    act_ssq_count += 1
                    if fine:
                        scalar.wait_ge(s_act_ssq, act_ssq_count)
                        scalar.activation(
                            nrm[:, k : k + 1], ssq[:, k : k + 1], Sqrt,
                            bias=eps_bias,
                        ).then_inc(s_act_nrm, 1)
                        nrm_count += 1
                if fine:
                    k = t * GQ + 3
                    scalar.wait_ge(s_dve_ssq, t + 1)
                    scalar.activation(
                        nrm[:, k : k + 1], ssq[:, k : k + 1], Sqrt,
                        bias=eps_bias,
                    ).then_inc(s_act_nrm, 1)
                    nrm_count += 1
                else:
                    scalar.wait_ge(s_act_ssq, act_ssq_count)
                    scalar.wait_ge(s_dve_ssq, t + 1)
                    scalar.activation(
                        nrm[:, t * GQ : (t + 1) * GQ],
                        ssq[:, t * GQ : (t + 1) * GQ],
                        Sqrt,
                        bias=eps_bias,
                    ).then_inc(s_act_nrm, 1)
                    nrm_count += 1
                if t == 4:
                    scalar.wait_ge(s_pe_mm, 16)
                    scalar.activation(
                        outA[:, 0, 0:512], ps[(0, 0, 0)][:, :], CopyF
                    ).then_inc(s_act_out, 1)
                    scalar.activation(
                        outA[:, 0, 512:1024], ps[(0, 0, 1)][:, :], CopyF
                    ).then_inc(s_act_out, 1)
            scalar.wait_ge(s_pe_mm, N_SUB)
            scalar.activation(
                outB[:, 0, 0:512], ps[(1, 0, 0)][:, :], CopyF
            ).then_inc(s_act_out, 1)
            scalar.activation(
                outB[:, 0, 512:1024], ps[(1, 0, 1)][:, :], CopyF
            ).then_inc(s_act_out, 1)

        @block.vector
        def _(vector):
            tick = 0

            def chain(ins):
                nonlocal tick
                ins.then_inc(s_dve, 1)
                tick += 1
                return tick

            vector.wait_ge(s_aux, 16)
            vector.wait_ge(s_lab, 16)
            chain(vector.tensor_copy(auxb[:, :], auxs[:, 0:C]))

            def do_group_chain(tg, fine_sub=None):
                rng = range(GQ) if fine_sub is None else (fine_sub,)
                if fine_sub is None:
                    vector.wait_ge(s_act_nrm, tg + 1)
                    sl = slice(tg * GQ, (tg + 1) * GQ)
                    chain(vector.reciprocal(rr[:, sl], nrm[:, sl]))
                else:
                    vector.wait_ge(s_act_nrm, N_GROUPS - 1 + fine_sub + 1)
                    k = tg * GQ + fine_sub
                    chain(vector.reciprocal(rr[:, k : k + 1], nrm[:, k : k + 1]))
                vector.wait_ge(s_dve, tick)
                for a in rng:
                    s = (tg, a)
                    k = ksub[s]
                    if is_f32[s]:
                        fi = fidx[s]
                        if fi >= OHF:
                            vector.wait_ge(s_pe_mm, ksub[fseq[fi - OHF]] + 1)
                        vector.tensor_scalar(
                            ohf[:, fi % OHF, :],
                            auxs[:, 0:C],
                            labf[:, tg, a : a + 1],
                            rr[:, k : k + 1],
                            mybir.AluOpType.is_equal,
                            mybir.AluOpType.mult,
                        ).then_inc(s_pl_oh, 1)
                    else:
                        bi = bidx[s]
                        if bi >= OHB:
                            vector.wait_ge(s_pe_mm, ksub[bseq[bi - OHB]] + 1)
                        vector.tensor_scalar(
                            ohb[:, bi % OHB, :],
                            auxb[:, :],
                            labf[:, tg, a : a + 1],
                            rr[:, k : k + 1],
                            mybir.AluOpType.is_equal,
                            mybir.AluOpType.mult,
                        ).then_inc(s_pl_oh, 1)
                for a in rng:
                    s = (tg, a)
                    if is_f32[s]:
                        continue
                    k = ksub[s]
                    if tg >= XSS:
                        vector.wait_ge(s_pe_mm, GQ * (tg - XSS + 1))
                    wait_x_lane(vector, tg, a)
                    vector.tensor_scalar(
                        xs[:, tg % XSS, a, :],
                        xt[:, tg % XSLOTS, a, :],
                        rr[:, k : k + 1],
                        None,
                        mybir.AluOpType.mult,
                    ).then_inc(s_xs, 1)

            for t in range(N_GROUPS):
                if t >= 1:
                    do_group_chain(t - 1)
                wait_x(vector, t, 1)
                k = t * GQ + 3
                vector.scalar_tensor_tensor(
                    vscr[:, :],
                    xt[:, t % XSLOTS, 3, :],
                    1.0,
                    xt[:, t % XSLOTS, 3, :],
                    mybir.AluOpType.mult,
                    mybir.AluOpType.mult,
                    accum_out=ssq[:, k : k + 1],
                ).then_inc(s_dve_ssq, 1)
                if t == 5:
                    vector.wait_ge(s_pe_mm, 16)
                    vector.tensor_copy(
                        outA[:, 1, 0:512], ps[(0, 1, 0)][:, :]
                    ).then_inc(s_dve_out, 1)
                    vector.tensor_copy(
                        outA[:, 1, 512:1024], ps[(0, 1, 1)][:, :]
                    ).then_inc(s_dve_out, 1)
            for a in range(GQ):
                do_group_chain(N_GROUPS - 1, fine_sub=a)
            vector.wait_ge(s_pe_mm, N_SUB)
            vector.tensor_copy(
                outB[:, 1, 0:512], ps[(1, 1, 0)][:, :]
            ).then_inc(s_dve_out, 1)
            vector.tensor_copy(
                outB[:, 1, 512:1024], ps[(1, 1, 1)][:, :]
            ).then_inc(s_dve_out, 1)

        @block.tensor
        def _(tensor):
            tensor.wait_ge(s_aux, 16)
            tensor.wait_ge(s_dve, 1)
            for _ in range(N_WARM):
                tensor.matmul(
                    ps[(1, 0, 0)][:, 0:256],
                    auxb[:, 0:128],
                    auxb[:, :],
                    start=True,
                    stop=True,
                )
            nxs = 0
            for t in range(N_GROUPS):
                for a in range(GQ):
                    s = (t, a)
                    k = ksub[s]
                    h = 0 if t < N_GROUPS // 2 else 1
                    first = k % 16 == 0
                    last = k % 16 == 15
                    tensor.wait_ge(s_pl_oh, k + 1)
                    if is_f32[s]:
                        wait_x_lane(tensor, t, a)
                        w = ohf[:, fidx[s] % OHF, :]
                        mv = xt[:, t % XSLOTS, a, :]
                    else:
                        tensor.wait_ge(s_xs, n_scales_upto[s])
                        w = ohb[:, bidx[s] % OHB, :]
                        mv = xs[:, t % XSS, a, :]
                    i = None
                    for mi in range(2):
                        for ni in range(2):
                            i = tensor.matmul(
                                ps[(h, mi, ni)][:, :],
                                w[:, mi * 128 : (mi + 1) * 128],
                                mv[:, ni * 512 : (ni + 1) * 512],
                                start=first,
                                stop=last,
                            )
                    i.then_inc(s_pe_mm, 1)

    return nc


def _norm_rows(x):
    x = x.astype(np.float64)
    n = np.sqrt((x * x).sum(axis=-1, keepdims=True))
    return x / np.maximum(n, EPS)


def _host_finish(feats, labels, S):
    b, d = feats.shape
    counts = np.bincount(labels, minlength=C)
    n = counts.astype(np.float64)
    mask = n > 1.0
    normS2 = (S * S).sum(axis=1)
    term1 = float(((n - normS2 / np.maximum(n, 1.0)) * mask).sum())

    nc_of_row = counts[labels]
    rows = np.nonzero(np.arange(b) < nc_of_row)[0]
    corr = 0.0
    if rows.size:
        order = np.argsort(labels, kind="stable")
        cls_sorted = labels[order]
        starts = np.searchsorted(cls_sorted, np.arange(C))
        need = set()
        for i in rows:
            c = int(labels[i])
            if counts[c] <= 1:
                continue
            k = int(order[starts[c] + i])
            need.add(int(i))
            need.add(k)
        need = sorted(need)
        fcache = {i: _norm_rows(feats[i]) for i in need}
        for i in rows:
            c = int(labels[i])
            n_c = float(counts[c])
            if n_c <= 1.0:
                continue
            k = int(order[starts[c] + i])
            f_i = fcache[int(i)]
            f_k = fcache[k]
            Sc = S[c]
            c_simple = Sc / n_c
            c_true = (Sc - f_k) / (n_c - 1.0)
            d_true = float(((f_i - c_true) ** 2).sum())
            d_simple = float(((f_i - c_simple) ** 2).sum())
            corr += d_true - d_simple

    total = term1 + corr
    return np.array(WEIGHT * total / (b * d), dtype=np.float32)


_nc_cache = None

TRACE = False
LAST_RESULTS = None


def _aux_input():
    a = np.zeros((P, C + 2), dtype=np.float32)
    a[:, :C] = np.arange(C, dtype=np.float32)[None, :]
    a[:, C + 1] = 1e-20
    return a


def kernel(features, labels):
    global _nc_cache, LAST_RESULTS
    feats = np.ascontiguousarray(np.asarray(features, dtype=np.float32))
    labs = np.ascontiguousarray(np.asarray(labels, dtype=np.int32))
    assert feats.shape == (B, D) and labs.shape == (B,)
    labs_f = labs.astype(np.float32)
    aux = _aux_input()
    if _nc_cache is None:
        _nc_cache = build_nc()
    in_maps = [
        {
            "x": feats[m * BS : (m + 1) * BS],
            "labf": labs_f[m * BS : (m + 1) * BS],
            "aux": aux,
        }
        for m in range(M_CORES)
    ]
    res = run_bass_kernel_spmd(
        _nc_cache, in_maps, core_ids=list(range(M_CORES)), trace=TRACE
    )
    LAST_RESULTS = res
    S = np.zeros((C, D), np.float64)
    for r in res.results:
        S += r["sumsA"].astype(np.float64)
        S += r["sumsB"].astype(np.float64)
    return _host_finish(feats, labs, S)


# revision 8
# speedup vs baseline: 1.2549x; 1.2549x over previous
import os
import sys
from contextlib import ExitStack

import numpy as np

for _p in ("/opt/trn_rl_repo", "/root/.axon_site/_ro/trn_rl_repo"):
    if os.path.isdir(_p) and _p not in sys.path:
        sys.path.insert(0, _p)

import concourse.bass as bass
from concourse import mybir
from concourse.bass_utils import run_bass_kernel_spmd

B, D, C = 32768, 1024, 256
M_CORES = 8
BS = B // M_CORES
P = 128
GQ = 4
N_GROUPS = BS // (P * GQ)
N_SUB = N_GROUPS * GQ
WEIGHT = 0.0005
EPS = 1e-12

F32 = mybir.dt.float32
BF16 = mybir.dt.bfloat16

XSLOTS = 6
XSS = 4
OHB = 8
OHF = 4
N_WARM = 16


def _sub_kind(t, a):
    return a == 3 and t % 2 == 0


def _ssq_on_dve(t, a):
    return a == 3


def build_nc(bs=BS):
    Sq = mybir.ActivationFunctionType.Square
    Sqrt = mybir.ActivationFunctionType.Sqrt
    CopyF = mybir.ActivationFunctionType.Copy

    subs = [(t, a) for t in range(N_GROUPS) for a in range(GQ)]
    is_f32 = {s: _sub_kind(*s) for s in subs}
    bidx, fidx = {}, {}
    bseq, fseq = [], []
    for s in subs:
        if is_f32[s]:
            fidx[s] = len(fseq)
            fseq.append(s)
        else:
            bidx[s] = len(bseq)
            bseq.append(s)
    ksub = {s: s[0] * GQ + s[1] for s in subs}
    n_casts_upto = {}
    cnt = 0
    for s in subs:
        if not is_f32[s]:
            cnt += 1
        n_casts_upto[s] = cnt

    nc = bass.Bass()
    x = nc.declare_dram_parameter("x", [bs, D], F32, isOutput=False)
    lab = nc.declare_dram_parameter("labf", [bs], F32, isOutput=False)
    aux = nc.declare_dram_parameter("aux", [P, C + 2], F32, isOutput=False)
    sumsA = nc.declare_dram_parameter("sumsA", [C, D], BF16, isOutput=True)
    sumsB = nc.declare_dram_parameter("sumsB", [C, D], BF16, isOutput=True)

    with ExitStack() as stk:
        en = stk.enter_context
        xt = en(nc.sbuf_tensor([P, XSLOTS, GQ, D], F32))
        xs = en(nc.sbuf_tensor([P, XSS, 4, D], BF16))
        sqscr = en(nc.sbuf_tensor([P, D], BF16))
        vscr = en(nc.sbuf_tensor([P, D], BF16))
        auxs = en(nc.sbuf_tensor([P, C + 2], F32))
        auxb = en(nc.sbuf_tensor([P, C], BF16))
        labf = en(nc.sbuf_tensor([P, N_GROUPS, GQ], F32))
        ssq = en(nc.sbuf_tensor([P, N_SUB], F32))
        nrm = en(nc.sbuf_tensor([P, N_SUB], F32))
        rr = en(nc.sbuf_tensor([P, N_SUB], F32))
        ohb = en(nc.sbuf_tensor([P, OHB, C], BF16))
        ohf = en(nc.sbuf_tensor([P, OHF, C], F32))
        outA = en(nc.sbuf_tensor([P, 2, D], BF16))
        outB = en(nc.sbuf_tensor([P, 2, D], BF16))
        dum = en(nc.sbuf_tensor([P, 2], F32))
        ps = {}
        for h in range(2):
            for mi in range(2):
                for ni in range(2):
                    ps[(h, mi, ni)] = en(
                        nc.psum_tensor(f"ps_{h}{mi}{ni}", [P, 512], F32)
                    )

        s_aux = en(nc.semaphore("s_aux"))
        s_lab = en(nc.semaphore("s_lab"))
        s_x0 = [en(nc.semaphore(f"s_x0_{a}")) for a in range(GQ)]
        s_x = [
            [en(nc.semaphore(f"s_x_{t}_{h}")) for h in range(2)]
            for t in range(1, N_GROUPS)
        ]
        s_act_ssq = en(nc.semaphore("s_act_ssq"))
        s_dve_ssq = en(nc.semaphore("s_dve_ssq"))
        s_act_nrm = en(nc.semaphore("s_act_nrm"))
        s_dve = en(nc.semaphore("s_dve"))
        s_pl_oh = en(nc.semaphore("s_pl_oh"))
        s_xs = en(nc.semaphore("s_xs"))
        s_pe_mm = en(nc.semaphore("s_pe_mm"))
        s_act_out = en(nc.semaphore("s_act_out"))
        s_dve_out = en(nc.semaphore("s_dve_out"))
        s_dma_out = en(nc.semaphore("s_dma_out"))
        block = en(nc.Block(no_gpsimd_drain=True))

        def wait_x(eng, t, half):
            if t == 0:
                eng.wait_ge(s_x0[2 * half + 1], 16)
            else:
                eng.wait_ge(s_x[t - 1][half], 16)

        def wait_x_lane(eng, t, a):
            if t == 0:
                eng.wait_ge(s_x0[a], 16)
            else:
                eng.wait_ge(s_x[t - 1][a // 2], 16)

        @block.sync
        def _(sync):
            sync.dma_start(out=auxs[:, :], in_=aux[:, :]).then_inc(s_aux, 16)
            src0 = x[0 : P * GQ, :].rearrange("(p g) d -> p g d", p=P)
            for a in range(GQ):
                sync.dma_start(out=xt[:, 0, a, :], in_=src0[:, a, :]).then_inc(
                    s_x0[a], 16
                )
            sync.dma_start(
                out=labf[:, :, :],
                in_=lab[0:bs].rearrange("(t p a) -> p t a", t=N_GROUPS, p=P, a=GQ),
            ).then_inc(s_lab, 16)
            for t in range(1, N_GROUPS):
                if t >= XSLOTS:
                    sync.wait_ge(s_pe_mm, GQ * (t - XSLOTS + 1))
                src = x[t * P * GQ : (t + 1) * P * GQ, :].rearrange(
                    "(p g) d -> p g d", p=P
                )
                sync.dma_start(
                    out=xt[:, t % XSLOTS, 0:2, :], in_=src[:, 0:2, :]
                ).then_inc(s_x[t - 1][0], 16)
                sync.dma_start(
                    out=xt[:, t % XSLOTS, 2:4, :], in_=src[:, 2:4, :]
                ).then_inc(s_x[t - 1][1], 16)
            sync.wait_ge(s_act_out, 2)
            sync.dma_start(out=sumsA[0:128, :], in_=outA[:, 0, :]).then_inc(
                s_dma_out, 16
            )
            sync.wait_ge(s_act_out, 4)
            sync.dma_start(out=sumsA[128:256, :], in_=outA[:, 1, :]).then_inc(
                s_dma_out, 16
            )
            sync.wait_ge(s_act_out, 6)
            sync.dma_start(out=sumsB[0:128, :], in_=outB[:, 0, :]).then_inc(
                s_dma_out, 16
            )
            sync.wait_ge(s_dve_out, 2)
            sync.dma_start(out=sumsB[128:256, :], in_=outB[:, 1, :]).then_inc(
                s_dma_out, 16
            )
            sync.wait_ge(s_dma_out, 64)

        @block.scalar
        def _(scalar):
            scalar.activation(dum[:, 0:1], dum[:, 1:2], Sq, bias=0.0)
            scalar.wait_ge(s_aux, 16)
            eps_bias = auxs[:, C + 1 : C + 2]
            act_ssq_count = 0
            nrm_count = 0
            for t in range(N_GROUPS):
                fine = t == N_GROUPS - 1
                acts = [a for a in range(GQ) if not _ssq_on_dve(t, a)]
                for a in acts:
                    k = t * GQ + a
                    wait_x_lane(scalar, t, a)
                    scalar.activation(
                        sqscr[:, :],
                        xt[:, t % XSLOTS, a, :],
                        Sq,
                        bias=0.0,
                        accum_out=ssq[:, k : k + 1],
                    ).then_inc(s_act_ssq, 1)
                    act_ssq_count += 1
                    if fine:
                        scalar.wait_ge(s_act_ssq, act_ssq_count)
                        scalar.activation(
                            nrm[:, k : k + 1], ssq[:, k : k + 1], Sqrt,
                            bias=eps_bias,
                        ).then_inc(s_act_nrm, 1)
                        nrm_count += 1
                if fine:
                    k = t * GQ + 3
                    scalar.wait_ge(s_dve_ssq, t + 1)
                    scalar.activation(
                        nrm[:, k : k + 1], ssq[:, k : k + 1], Sqrt,
                        bias=eps_bias,
                    ).then_inc(s_act_nrm, 1)
                    nrm_count += 1
                else:
                    scalar.wait_ge(s_act_ssq, act_ssq_count)
                    scalar.wait_ge(s_dve_ssq, t + 1)
                    scalar.activation(
                        nrm[:, t * GQ : (t + 1) * GQ],
                        ssq[:, t * GQ : (t + 1) * GQ],
                        Sqrt,
                        bias=eps_bias,
                    ).then_inc(s_act_nrm, 1)
                    nrm_count += 1
                if t == 4:
                    scalar.wait_ge(s_pe_mm, 16)
                    for mi in range(2):
                        for ni in range(2):
                            scalar.activation(
                                outA[:, mi, ni * 512 : (ni + 1) * 512],
                                ps[(0, mi, ni)][:, :],
                                CopyF,
                            ).then_inc(s_act_out, 1)
            scalar.wait_ge(s_pe_mm, N_SUB)
            scalar.activation(
                outB[:, 0, 0:512], ps[(1, 0, 0)][:, :], CopyF
            ).then_inc(s_act_out, 1)
            scalar.activation(
                outB[:, 0, 512:1024], ps[(1, 0, 1)][:, :], CopyF
            ).then_inc(s_act_out, 1)

        @block.vector
        def _(vector):
            tick = 0

            def chain(ins):
                nonlocal tick
                ins.then_inc(s_dve, 1)
                tick += 1
                return tick

            vector.wait_ge(s_aux, 16)
            vector.wait_ge(s_lab, 16)
            chain(vector.tensor_copy(auxb[:, :], auxs[:, 0:C]))

            def do_group_chain(tg, fine_sub=None):
                rng = range(GQ) if fine_sub is None else (fine_sub,)
                if fine_sub is None:
                    vector.wait_ge(s_act_nrm, tg + 1)
                    sl = slice(tg * GQ, (tg + 1) * GQ)
                    chain(vector.reciprocal(rr[:, sl], nrm[:, sl]))
                else:
                    vector.wait_ge(s_act_nrm, N_GROUPS - 1 + fine_sub + 1)
                    k = tg * GQ + fine_sub
                    chain(vector.reciprocal(rr[:, k : k + 1], nrm[:, k : k + 1]))
                vector.wait_ge(s_dve, tick)
                for a in rng:
                    s = (tg, a)
                    k = ksub[s]
                    if is_f32[s]:
                        fi = fidx[s]
                        if fi >= OHF:
                            vector.wait_ge(s_pe_mm, ksub[fseq[fi - OHF]] + 1)
                        vector.tensor_scalar(
                            ohf[:, fi % OHF, :],
                            auxs[:, 0:C],
                            labf[:, tg, a : a + 1],
                            rr[:, k : k + 1],
                            mybir.AluOpType.is_equal,
                            mybir.AluOpType.mult,
                        ).then_inc(s_pl_oh, 1)
                    else:
                        bi = bidx[s]
                        if bi >= OHB:
                            vector.wait_ge(s_pe_mm, ksub[bseq[bi - OHB]] + 1)
                        vector.tensor_scalar(
                            ohb[:, bi % OHB, :],
                            auxb[:, :],
                            labf[:, tg, a : a + 1],
                            rr[:, k : k + 1],
                            mybir.AluOpType.is_equal,
                            mybir.AluOpType.mult,
                        ).then_inc(s_pl_oh, 1)
            for t in range(N_GROUPS):
                if t >= 1:
                    do_group_chain(t - 1)
                wait_x(vector, t, 1)
                k = t * GQ + 3
                vector.scalar_tensor_tensor(
                    vscr[:, :],
                    xt[:, t % XSLOTS, 3, :],
                    1.0,
                    xt[:, t % XSLOTS, 3, :],
                    mybir.AluOpType.mult,
                    mybir.AluOpType.mult,
                    accum_out=ssq[:, k : k + 1],
                ).then_inc(s_dve_ssq, 1)
                if t >= XSS:
                    vector.wait_ge(s_pe_mm, GQ * (t - XSS + 1))
                first_low = True
                for a in range(GQ):
                    if is_f32[(t, a)]:
                        continue
                    if a < 2 and first_low:
                        wait_x_lane(vector, t, a)
                        first_low = False
                    vector.tensor_copy(
                        xs[:, t % XSS, a, :], xt[:, t % XSLOTS, a, :]
                    ).then_inc(s_xs, 1)
            for a in range(GQ):
                do_group_chain(N_GROUPS - 1, fine_sub=a)
            vector.wait_ge(s_pe_mm, N_SUB)
            vector.tensor_copy(
                outB[:, 1, 0:512], ps[(1, 1, 0)][:, :]
            ).then_inc(s_dve_out, 1)
            vector.tensor_copy(
                outB[:, 1, 512:1024], ps[(1, 1, 1)][:, :]
            ).then_inc(s_dve_out, 1)

        @block.tensor
        def _(tensor):
            tensor.wait_ge(s_aux, 16)
            tensor.wait_ge(s_dve, 1)
            for _ in range(N_WARM):
                tensor.matmul(
                    ps[(1, 0, 0)][:, 0:256],
                    auxb[:, 0:128],
                    auxb[:, :],
                    start=True,
                    stop=True,
                )
            nxs = 0
            for t in range(N_GROUPS):
                for a in range(GQ):
                    s = (t, a)
                    k = ksub[s]
                    h = 0 if t < N_GROUPS // 2 else 1
                    first = k % 16 == 0
                    last = k % 16 == 15
                    tensor.wait_ge(s_pl_oh, k + 1)
                    if is_f32[s]:
                        wait_x_lane(tensor, t, a)
                        w = ohf[:, fidx[s] % OHF, :]
                        mv = xt[:, t % XSLOTS, a, :]
                    else:
                        tensor.wait_ge(s_xs, n_casts_upto[s])
                        w = ohb[:, bidx[s] % OHB, :]
                        mv = xs[:, t % XSS, a, :]
                    i = None
                    for mi in range(2):
                        for ni in range(2):
                            i = tensor.matmul(
                                ps[(h, mi, ni)][:, :],
                                w[:, mi * 128 : (mi + 1) * 128],
                                mv[:, ni * 512 : (ni + 1) * 512],
                                start=first,
                                stop=last,
                            )
                    i.then_inc(s_pe_mm, 1)

    return nc


def _norm_rows(x):
    x = x.astype(np.float64)
    n = np.sqrt((x * x).sum(axis=-1, keepdims=True))
    return x / np.maximum(n, EPS)


def _host_finish(feats, labels, S):
    b, d = feats.shape
    counts = np.bincount(labels, minlength=C)
    n = counts.astype(np.float64)
    mask = n > 1.0
    normS2 = (S * S).sum(axis=1)
    term1 = float(((n - normS2 / np.maximum(n, 1.0)) * mask).sum())

    nc_of_row = counts[labels]
    rows = np.nonzero(np.arange(b) < nc_of_row)[0]
    corr = 0.0
    if rows.size:
        order = np.argsort(labels, kind="stable")
        cls_sorted = labels[order]
        starts = np.searchsorted(cls_sorted, np.arange(C))
        need = set()
        for i in rows:
            c = int(labels[i])
            if counts[c] <= 1:
                continue
            k = int(order[starts[c] + i])
            need.add(int(i))
            need.add(k)
        need = sorted(need)
        fcache = {i: _norm_rows(feats[i]) for i in need}
        for i in rows:
            c = int(labels[i])
            n_c = float(counts[c])
            if n_c <= 1.0:
                continue
            k = int(order[starts[c] + i])
            f_i = fcache[int(i)]
            f_k = fcache[k]
            Sc = S[c]
            c_simple = Sc / n_c
            c_true = (Sc - f_k) / (n_c - 1.0)
            d_true = float(((f_i - c_true) ** 2).sum())
            d_simple = float(((f_i - c_simple) ** 2).sum())
            corr += d_true - d_simple

    total = term1 + corr
    return np.array(WEIGHT * total / (b * d), dtype=np.float32)


_nc_cache = None

TRACE = False
LAST_RESULTS = None


def _aux_input():
    a = np.zeros((P, C + 2), dtype=np.float32)
    a[:, :C] = np.arange(C, dtype=np.float32)[None, :]
    a[:, C + 1] = 1e-20
    return a


def kernel(features, labels):
    global _nc_cache, LAST_RESULTS
    feats = np.ascontiguousarray(np.asarray(features, dtype=np.float32))
    labs = np.ascontiguousarray(np.asarray(labels, dtype=np.int32))
    assert feats.shape == (B, D) and labs.shape == (B,)
    labs_f = labs.astype(np.float32)
    aux = _aux_input()
    if _nc_cache is None:
        _nc_cache = build_nc()
    in_maps = [
        {
            "x": feats[m * BS : (m + 1) * BS],
            "labf": labs_f[m * BS : (m + 1) * BS],
            "aux": aux,
        }
        for m in range(M_CORES)
    ]
    res = run_bass_kernel_spmd(
        _nc_cache, in_maps, core_ids=list(range(M_CORES)), trace=TRACE
    )
    LAST_RESULTS = res
    S = np.zeros((C, D), np.float64)
    for r in res.results:
        S += r["sumsA"].astype(np.float64)
        S += r["sumsB"].astype(np.float64)
    return _host_finish(feats, labs, S)
